# revision 33
# baseline (speedup 1.0000x reference)
"""Multi-head causal attention (B=2, S=2048, D=1024, H=16, DK=DV=64) on 8 Trainium2
NeuronCores.

Sharding: 2-way batch x 4-way head-group. Core i handles batch i//4 and heads
[4*(i%4), 4*(i%4)+4). Each core projects q/k/v for its head group, runs causal
attention, and computes a partial output projection through its row-block of Wo.
The 4 partial outputs per batch are summed on the host (the all-reduce of the
row-sharded Wo output).

Design notes (all-bf16, PE-saturating schedule):
- Everything is bf16 on the wire and in SBUF (host casts); psum stays fp32.
- q/k live transposed per head ([dk, s]); scores are computed transposed
  ([s_k, s_q]). v is projected directly in natural layout ([s_k, dv]) by using
  the x tile as the matmul stationary operand, so no PE transposes are needed.
- Attention runs on k-tile PAIRS: two score matmuls write the two halves of
  one 2-bank psum tile [128, 1024]; a single Scalar-engine exp covers the
  pair. Scores, exp, and attn@v matmuls are trimmed to the causally valid
  column range, so no zero-fill of masked regions is needed (the exp of stale
  psum garbage is bounded and never read). attn@v matmuls trail their scores
  by three pairs so the exp latency is fully hidden.
- PSUM (8 banks) is one shared 3-deep ring of 2-bank tiles serving score
  pairs and all filler matmuls, plus two 1-bank attn@v accumulators.
- The denominator comes free from an all-ones column appended to v; fast
  approximate reciprocal, then a GpSimd partition_broadcast replicates it
  across partitions (no PE involvement).
- The Scalar engine does (almost) only exp; psum->sbuf copies run on Vector;
  DMA triggers on Sync/GpSimd (plus the idle ACT queue during the prologue,
  which streams the weight groups v-first while x halves stream on
  sync/gpsimd).
- PE "filler" work (output projection + normalize of the previous chunk,
  projections of the next x-chunk) is drained between attention pairs with
  debt-based pacing so the PE pipeline never drains and the tensor engine
  stays at its top p-state clock.
"""
import sys

sys.path.insert(0, "/opt/trn_rl_repo")
import numpy as np

B, S, D = 2, 2048, 1024
H, DK, DV = 16, 64, 64
NCORES = 8
HG = 4          # head-group cores per batch
HPC = H // HG   # heads per core
HDC = HPC * DK  # 256 projection cols per core
P = 128         # partitions
CH = 512        # q-chunk size
XC = 512        # x-stream chunk for projections
VW = DV + 1     # v_aug width per head: v cols then a ones col


def build(nc, tile, mybir, s=S, d=D):
    F32 = mybir.dt.float32
    BF16 = mybir.dt.bfloat16
    Exp = mybir.ActivationFunctionType.Exp
    xc = min(XC, s)    # x stream chunk
    nch = s // CH      # q-chunks
    nst = s // P       # s-tiles (also k-tiles)
    nd = d // P        # d-tiles
    nxc = s // xc      # x stream chunks
    nm = HDC // P      # head-pair tiles
    cpx = xc // CH     # q-chunks per x chunk

    xqT = nc.dram_tensor("xqT", [d, s], BF16, kind="ExternalInput").ap()
    xkT = nc.dram_tensor("xkT", [d, s], BF16, kind="ExternalInput").ap()
    xvT = nc.dram_tensor("xvT", [d, s], BF16, kind="ExternalInput").ap()
    wqkv = nc.dram_tensor("wqkv", [d, 3 * HDC], BF16, kind="ExternalInput").ap()
    wo = nc.dram_tensor("wo", [HDC, d], BF16, kind="ExternalInput").ap()
    maskA = nc.dram_tensor("maskA", [P, P], BF16, kind="ExternalInput").ap()
    vinit = nc.dram_tensor("vinit", [P, HPC * VW], BF16, kind="ExternalInput").ap()
    out = nc.dram_tensor("out", [s, d], BF16, kind="ExternalOutput").ap()

    with tile.TileContext(nc) as tc:
        from contextlib import ExitStack
        with ExitStack() as ctx:
            wp = ctx.enter_context(tc.tile_pool(name="wp", bufs=1))
            xp = ctx.enter_context(tc.tile_pool(name="xp", bufs=4))
            per = ctx.enter_context(tc.tile_pool(name="per", bufs=1))
            ep = ctx.enter_context(tc.tile_pool(name="ep", bufs=8))
            sp = ctx.enter_context(tc.tile_pool(name="sp", bufs=2))
            obp = ctx.enter_context(tc.tile_pool(name="obp", bufs=3))
            # psum: 8 banks of [128, 512]f32. One shared ring of 3 2-bank
            # tiles serves score pairs AND filler matmuls (projections /
            # output projection), plus 2 single-bank ov accumulators.
            sc_ps = ctx.enter_context(tc.tile_pool(name="sc_ps", bufs=3, space="PSUM"))
            ov_ps = ctx.enter_context(tc.tile_pool(name="ov_ps", bufs=2, space="PSUM"))

            def ring_tile():
                return sc_ps.tile([P, 2 * CH], F32, name="sc2", tag="sc")

            # --- tiles ---
            wall = wp.tile([P, nd * 3 * HDC], BF16, name="wall")
            wv3 = wall[:].rearrange("p (t c) -> p t c", t=nd)
            wq_t = [wv3[:, i, 0:HDC] for i in range(nd)]
            wk_t = [wv3[:, i, HDC:2 * HDC] for i in range(nd)]
            wv_t = [wv3[:, i, 2 * HDC:3 * HDC] for i in range(nd)]
            wo_t = [wp.tile([P, d], BF16, name=f"wo{i}") for i in range(nm)]
            mA = wp.tile([P, P], BF16, name="mA")
            vtmp = wp.tile([P, HPC * VW], BF16, name="vtmp")

            # --- persistent activations ---
            qT = [per.tile([P, s], BF16, name=f"qT{m}") for m in range(nm)]
            kTt = [per.tile([P, s], BF16, name=f"kT{m}") for m in range(nm)]
            oT = [per.tile([P, s], BF16, name=f"oT{m}") for m in range(nm)]
            vaug = [per.tile([P, HPC * VW], BF16, name=f"vaug{t}")
                    for t in range(nst)]

            # x chunks: [128, nd, xc] (d-tile index on the middle axis)
            def load_x(xT, sc, three_way=False):
                xt = xp.tile([P, nd * xc], BF16, name="xt", tag="xt")
                xv = xt[:].rearrange("p (t c) -> p t c", t=nd)
                src = xT[:, sc * xc:(sc + 1) * xc].rearrange(
                    "(t p) c -> p t c", p=P)
                h0 = nd // 2
                if three_way:
                    nc.sync.dma_start(xv[:, 0:h0], src[:, 0:h0])
                    nc.gpsimd.dma_start(xv[:, h0:nd], src[:, h0:nd])
                else:
                    # mid-stream: keep the Pool queue free for broadcasts
                    nc.sync.dma_start(xv[:, 0:h0], src[:, 0:h0])
                    nc.sync.dma_start(xv[:, h0:nd], src[:, h0:nd])
                return xt

            def load_consts():
                # small constants; weight groups are issued by the prologue
                nc.scalar.dma_start(vtmp[:], vinit[:, :])
                for t in range(nst):
                    nc.vector.tensor_copy(vaug[t][:], vtmp[:])
                for i in range(nm):
                    nc.scalar.dma_start(wo_t[i][:], wo[i * P:(i + 1) * P, :])


            def load_w_group(g):
                wsrc = wqkv[:, :].rearrange("(t p) c -> p t c", p=P)
                nc.scalar.dma_start(wv3[:, :, g * HDC:(g + 1) * HDC],
                                    wsrc[:, :, g * HDC:(g + 1) * HDC])

            # --- filler machinery: closures that emit PE-centric work ---
            fillers = []

            def drain_fillers(k):
                for _ in range(k):
                    if fillers:
                        fillers.pop(0)()

            def proj_unit(xt, w_t, dstT, sc, m, n2, eng):
                """dstT[m][:, sc*xc + n2*512 ...] via 8 accumulated matmuls."""
                def emit():
                    xv = xt[:].rearrange("p (t c) -> p t c", t=nd)
                    pp = ring_tile()
                    for dd in range(nd):
                        nc.tensor.matmul(
                            pp[:, 0:512], w_t[dd][:, m * P:(m + 1) * P],
                            xv[:, dd, n2 * 512:(n2 + 1) * 512],
                            start=(dd == 0), stop=(dd == nd - 1))
                    dsl = dstT[m][:, sc * xc + n2 * 512:
                                  sc * xc + (n2 + 1) * 512]
                    eng.tensor_copy(dsl, pp[:, 0:512])
                return emit

            def vproj_unit(xt, sc, stl, eng):
                """vaug[sc*(xc//P) + stl] <- natural-layout v projection."""
                def emit():
                    xv = xt[:].rearrange("p (t c) -> p t c", t=nd)
                    vn = ring_tile()
                    for dd in range(nd):
                        nc.tensor.matmul(
                            vn[:, 0:HDC], xv[:, dd, stl * P:(stl + 1) * P],
                            wv_t[dd][:], start=(dd == 0), stop=(dd == nd - 1))
                    st = sc * (xc // P) + stl
                    dst = vaug[st][:].rearrange(
                        "p (h x) -> p h x", x=VW)[:, :, 0:DV]
                    src = vn[:, 0:HDC].rearrange("p (h x) -> p h x", x=DV)
                    eng.tensor_copy(dst, src)
                return emit

            def normalize_m(c, m, dpair):
                """Reciprocal + rank-1 broadcasts + oT scale for head pair m."""
                def emit():
                    F32 = mybir.dt.float32
                    for par in (0, 1):
                        rp = sp.tile([1, CH], F32, name=f"rp{par}",
                                     tag=f"rp{par}", bufs=2)
                        nc.vector.reciprocal_approx_fast(rp[:], dpair[par][:])
                        rpb = sp.tile([1, CH], BF16, name=f"rpb{par}",
                                      tag=f"rpb{par}", bufs=2)
                        nc.vector.tensor_copy(rpb[:], rp[:])
                        recT = sp.tile([P, CH], BF16, name=f"recT{par}",
                                       tag=f"recT{par}", bufs=2)
                        nc.gpsimd.partition_broadcast(recT[:], rpb[:])
                        sl = oT[m][par * DK:(par + 1) * DK,
                                   c * CH:(c + 1) * CH]
                        nc.vector.tensor_mul(
                            sl, sl, recT[par * DK:(par + 1) * DK, :])
                return emit

            def oproj_unit(st, n, eng, ob, obs):
                """ob[:, n*512...] = oT[:, st-tile].T @ wo[:, n*512...]."""
                def emit():
                    pp = ring_tile()
                    for m in range(nm):
                        nc.tensor.matmul(pp[:, 0:512],
                                         oT[m][:, st * P:(st + 1) * P],
                                         wo_t[m][:, n * 512:(n + 1) * 512],
                                         start=(m == 0), stop=(m == nm - 1))
                    eng.tensor_copy(ob[:, n * 512:(n + 1) * 512], pp[:, 0:512])
                    obs[0] += 1
                    if obs[0] == d // 512:
                        deng = (nc.sync, nc.gpsimd)[st % 2]
                        deng.dma_start(out[st * P:(st + 1) * P, :], ob[:])
                return emit

            def queue_oproj(c):
                for stl in range(CH // P):
                    st = c * (CH // P) + stl
                    ob = obp.tile([P, d], BF16, name="ob", tag="ob")
                    obs = [0]
                    for n in range(d // 512):
                        fillers.append(oproj_unit(st, n, nc.vector, ob, obs))

            def queue_projections(sc, consts_after_load=False):
                if consts_after_load:
                    # prologue: mask first (feeds the PE warmup), then weight
                    # groups on the idle ACT queue (v first) while x halves
                    # stream on sync/gpsimd
                    nc.scalar.dma_start(mA[:], maskA[:, :])
                    load_w_group(2)
                    xtv = load_x(xvT, sc, three_way=True)
                    load_w_group(1)
                    xtk = load_x(xkT, sc, three_way=True)
                    load_w_group(0)
                    xtq = load_x(xqT, sc, three_way=True)
                    load_consts()
                else:
                    xtv = load_x(xvT, sc)
                for stl in range(xc // P):
                    fillers.append(vproj_unit(xtv, sc, stl, nc.vector))
                if not consts_after_load:
                    xtk = load_x(xkT, sc)
                    xtq = load_x(xqT, sc)
                for m in range(nm):
                    for n2 in range(xc // 512):
                        fillers.append(proj_unit(xtk, wk_t, kTt, sc, m, n2,
                                                 nc.vector))
                        fillers.append(proj_unit(xtq, wq_t, qT, sc, m, n2,
                                                 nc.vector))

            def attention(h, c, dpair, pace):
                """Head h, q-chunk c: paired k-tiles, trimmed causal ranges."""
                mi, ri = h // 2, (h % 2) * DK
                nt = 4 * c + 4
                ov = ov_ps.tile([DV + 1, CH], F32, name="ov", tag="ov")
                qsl = qT[mi][ri:ri + DK, :]
                ksl = kTt[mi][ri:ri + DK, :]
                exs = []  # (ex, t0, lo_a, lo_b) pending av pairs
                for pt in range(nt // 2):
                    t0 = 2 * pt
                    lo = [max(t0 - 4 * c, 0) * P, max(t0 + 1 - 4 * c, 0) * P]
                    sc2 = ring_tile()
                    for i in (0, 1):
                        t = t0 + i
                        nc.tensor.matmul(
                            sc2[:, i * CH + lo[i]:(i + 1) * CH],
                            ksl[:, t * P:(t + 1) * P],
                            qsl[:, c * CH + lo[i]:(c + 1) * CH],
                            start=True, stop=True)
                    ex = ep.tile([P, 2 * CH], BF16, name="ex", tag="ex")
                    nc.scalar.activation(ex[:, lo[0]:], sc2[:, lo[0]:], Exp)
                    for i in (0, 1):
                        t = t0 + i
                        if t - 4 * c >= 0:
                            nc.vector.tensor_mul(
                                ex[:, i * CH + lo[i]:i * CH + lo[i] + P],
                                ex[:, i * CH + lo[i]:i * CH + lo[i] + P],
                                mA[:])
                    # delay avs one pair so exp overlaps the next score pair
                    exs.append((ex, t0, lo))
                    if len(exs) == 4:
                        pace()
                        emit_avs(h, ov, exs.pop(0), False)
                while len(exs) > 1:
                    emit_avs(h, ov, exs.pop(0), False)
                emit_avs(h, ov, exs.pop(0), True)
                pace()
                # numerator -> oT (unnormalized); denominator from row DV
                nc.vector.tensor_copy(oT[mi][ri:ri + DK, c * CH:(c + 1) * CH],
                                      ov[0:DV, :])
                nc.vector.tensor_copy(dpair[h % 2][:], ov[DV:DV + 1, :])

            def emit_avs(h, ov, exent, last):
                ex, t0, lo = exent
                for i in (0, 1):
                    t = t0 + i
                    nc.tensor.matmul(
                        ov[:, lo[i]:CH],
                        vaug[t][:, h * VW:(h + 1) * VW],
                        ex[:, i * CH + lo[i]:(i + 1) * CH],
                        start=(t == 0), stop=(last and i == 1))

            # --- main pipeline ---
            queue_projections(0, consts_after_load=True)
            # prologue: project v + the m=0 tiles of k/q; the m=1 tiles
            # drain during heads 0-1 of the first chunk
            drain_fillers(len(fillers) - 2)
            prev = None
            for sc in range(nxc):
                for c in range(sc * cpx, (sc + 1) * cpx):
                    if c == sc * cpx and c > 0:
                        # everything queued so far (incl. this x-chunk's
                        # projections) must precede this chunk's attention in
                        # the PE stream, else the engine FIFOs deadlock
                        drain_fillers(len(fillers))
                    if prev is not None:
                        queue_oproj(prev)
                    if c == sc * cpx + cpx - 1 and sc + 1 < nxc:
                        queue_projections(sc + 1)
                    npairs = 2 * c + 2
                    nslots = HPC * npairs
                    n0 = len(fillers) + 2  # +2: normalize fillers added below
                    slot = [0, 0]

                    def pace():
                        slot[0] += 1
                        tgt = slot[0] * n0 // nslots
                        drain_fillers(tgt - slot[1])
                        slot[1] = tgt

                    for h in range(HPC):
                        if h % 2 == 0:
                            dpair = [sp.tile([1, CH], F32, name=f"dst{par}",
                                             tag=f"dst{par}", bufs=2)
                                     for par in (0, 1)]
                        attention(h, c, dpair, pace)
                        if h % 2 == 1:
                            fillers.append(normalize_m(c, h // 2, dpair))
                    prev = c
            queue_oproj(prev)
            drain_fillers(len(fillers))
    nc.compile()
    return nc


_NC_CACHE = {}
LAST_RESULT = None


def _get_nc(s=S, d=D):
    key = (s, d)
    if key not in _NC_CACHE:
        import concourse.tile as tile
        import concourse.mybir as mybir
        from concourse import bacc
        nc = bacc.Bacc("TRN2", target_bir_lowering=False, num_devices=NCORES)
        _NC_CACHE[key] = build(nc, tile, mybir, s=s, d=d)
    return _NC_CACHE[key]


def make_masks():
    import ml_dtypes
    i = np.arange(P)[:, None]
    j = np.arange(P)[None, :]
    maskA = (j >= i).astype(ml_dtypes.bfloat16)
    vinit = np.zeros((P, HPC * VW), dtype=ml_dtypes.bfloat16)
    for h in range(HPC):
        vinit[:, h * VW + DV] = 1
    return maskA, vinit


def kernel(Q, K, V, Wq, Wk, Wv, Wo):
    import ml_dtypes
    from concourse.bass_utils import run_bass_kernel_spmd

    BF = ml_dtypes.bfloat16
    Q = np.asarray(Q, dtype=np.float32)
    K = np.asarray(K, dtype=np.float32)
    V = np.asarray(V, dtype=np.float32)
    Wq = (np.asarray(Wq, dtype=np.float32)
          * np.float32(1.0 / np.sqrt(DK))).astype(BF)
    Wk = np.asarray(Wk, dtype=np.float32).astype(BF)
    Wv = np.asarray(Wv, dtype=np.float32).astype(BF)
    Wo = np.asarray(Wo, dtype=np.float32).astype(BF)

    QT = [np.ascontiguousarray(Q[b].T).astype(BF) for b in range(B)]
    KT = [np.ascontiguousarray(K[b].T).astype(BF) for b in range(B)]
    VT = [np.ascontiguousarray(V[b].T).astype(BF) for b in range(B)]
    maskA, vinit = make_masks()

    in_maps = []
    for core in range(NCORES):
        b, g = core // HG, core % HG
        cs = slice(g * HDC, (g + 1) * HDC)
        in_maps.append({
            "xqT": QT[b], "xkT": KT[b], "xvT": VT[b],
            "wqkv": np.ascontiguousarray(
                np.concatenate([Wq[:, cs], Wk[:, cs], Wv[:, cs]], axis=1)),
            "wo": np.ascontiguousarray(Wo[cs, :]),
            "maskA": maskA, "vinit": vinit,
        })

    nc = _get_nc()
    res = run_bass_kernel_spmd(nc, in_maps, core_ids=list(range(NCORES)))
    global LAST_RESULT
    LAST_RESULT = res

    acc = np.zeros((B, S, D), dtype=np.float64)
    for core in range(NCORES):
        acc[core // HG] += res.results[core]["out"].astype(np.float64)
    return acc.astype(np.float32)
